# revision 44
# baseline (speedup 1.0000x reference)
"""Trainium2 Bass kernel: gated MSA row attention (AlphaFold-style).

Shapes: q_data/k_data [1,128,256,256], bias [1,8,256,256], k_mask [1,128,256].
Sharding: data-parallel over the 128 sequences -> 16 per core on 8 cores.

v3 design notes (vs the v2 baseline, 179us -> ~152us):
- Bias delivery via exp-factoring: weights = exp(logits+mask) * exp(bias).
  exp(bias) is computed ONCE per core (4 ACTs); the per-sequence ident
  bias-broadcast matmuls (128 insts, ~27us of PE time) are gone, replaced
  by a bf16 VectorE multiply on the exp tiles (2x_1P DVE mode; NOT
  in-place — out==in0 drops the op to 1x on HW).
- Head block order POS=[0,2,4,6,1,3,5,7]: the 8 head matmuls of one
  (si,kc) go into ONE 4-bank PSUM tile as two quads, each quad hitting 4
  distinct row groups AND 4 distinct banks -> true 4-way tile
  concurrency (identity order put 2 concurrent streams on one PSUM bank
  -> hardware error; v2's order was bank-clean but only 2-way).
  Wo rows / Wg cols / bg are permuted on the host (pure layout) so the
  gate/out-proj algebra is unchanged.
- Software pipelining: each sequence's tail (wavg/denom/gate/out-proj)
  is issued during the NEXT sequence's logits phase, so the PE fills the
  exp-ACT latency instead of head-of-line blocking (engine queues are
  program-order FIFOs).
- Everything ships bf16 from the host (inputs, weights, bias): no
  on-device casts, half the DMA bytes, and bf16 matmuls stream 2x faster
  than fp32 (fp32_mode=HIGH measured ~1.3ns/col).
- The exp evacuation is split per 2-bank half (earlier PSUM release);
  input DMAs are merged to one descriptor per (seq, tensor).
- bo is added by a K=1 ones-row matmul into the out-proj PSUM group; the
  PSUM->SBUF output evacuation and one v-evacuation per pair run on
  ScalarE (AF.Copy) to offload VectorE, which is the pacing engine.
- GpSimd is left idle on purpose: its elementwise ops run at ~0.4x DVE
  AND poison concurrent DVE throughput via SBUF port contention
  (measured: gated-mul on GpSimd cost +16us end-to-end).
"""

import os
import sys
import numpy as np
from contextlib import ExitStack

sys.path.insert(0, "/opt/trn_rl_repo")

import concourse.bass as bass
import concourse.bacc as bacc
import concourse.mybir as mybir
from concourse import tile
from concourse.bass_utils import run_bass_kernel_spmd

NCORES = 8
S = 128
SS = S // NCORES          # 16 sequences per core
L = 256                   # residues (q and k length)
C = 256                   # channels
H = 8                     # heads
DK = 32                   # head dim
SCALE = 1.0 / np.sqrt(DK)
MASK_NEG = -30.0          # additive logit offset for masked keys

F32 = mybir.dt.float32
F32R = mybir.dt.float32r
BF16 = mybir.dt.bfloat16
U8 = mybir.dt.uint8
AF = mybir.ActivationFunctionType

# bf16 weight pack (DMA'd first; weights usable with no on-device casts)
OFF_WQ = 0
OFF_WK = OFF_WQ + 512
OFF_WV = OFF_WK + 512
OFF_WG = OFF_WV + 512
OFF_WO = OFF_WG + 512
OFF_BO1 = OFF_WO + 512     # bo on row 0, bf16
OFF_ID = OFF_BO1 + 256     # 128x128 identity, bf16 (PE bias broadcast)
NWPACK = OFF_ID + 128
# small f32 pack (bo2 broadcast for the no-BOMM path, bg)
OFF_BO2 = 0
OFF_BG = OFF_BO2 + 512
NPACK = OFF_BG + 2

# feature toggles for HW bisection (sim passes with all on)
USE_BF16_PROJ = bool(int(os.environ.get("K_BF16", "1")))   # bf16 inputs + weights
USE_BO_MM = bool(int(os.environ.get("K_BOMM", "1")))       # bo via K=1 matmul
HEADS_4WAY = bool(int(os.environ.get("K_H4", "1")))        # identity head order
GP_GATED = bool(int(os.environ.get("K_GPG", "0")))         # gated mul on gpsimd
INPLACE_EB = bool(int(os.environ.get("K_INPL", "0")))      # e2 *= ebb in place
EB_GP = int(os.environ.get("K_EBGP", "0"))                 # every Nth eb-mult on gpsimd (0=off)
# every Nth (s,kc) tile gets bias via PE ident-broadcast matmuls instead of
# the VectorE exp(bias) multiply (0=off): VectorE is the pacer, PE has slack
PE_BIAS = int(os.environ.get("K_PEB", "0"))

# head h -> exp/logits block POS[h]; HEAD_AT = inverse.
# 4-way mode: logits go to ONE 4-bank PSUM tile [128,2048]; blocks are
# issued as quads (0,2,4,6) then (1,3,5,7) so each concurrent stream has
# its own bank (2 streams draining one PSUM bank is a HW error) AND its
# own PE row group (r = 32*(HEAD_AT[b]%4) distinct within a quad).
# v2 order ([h0,h4|h1,h5|h2,h6|h3,h7]) pairs banks cleanly but only gets
# 2-way row concurrency.
if HEADS_4WAY:
    POS = [0, 2, 4, 6, 1, 3, 5, 7]
else:
    POS = [2 * (h % 4) + (h // 4) for h in range(8)]
HEAD_AT = [0] * 8
for _h in range(8):
    HEAD_AT[POS[_h]] = _h

# partition p of the gated/wavg col-block c holds hd = 32*HEAD_AT[2*(p//32)+c] + p%32
PERM_HD = [[32 * HEAD_AT[2 * (p // 32) + c] + (p % 32) for p in range(128)]
           for c in range(2)]

_CACHE = {}


def _build_nc():
    nc = bacc.Bacc()

    assert USE_BF16_PROJ, "f32r path removed: weights ship as bf16"
    in_dt = BF16
    xqT_e = nc.declare_dram_parameter("xqT", [SS, C, L], in_dt, isOutput=False)
    xkT_e = nc.declare_dram_parameter("xkT", [SS, C, L], in_dt, isOutput=False)
    maskT_e = nc.declare_dram_parameter("maskT", [128, 2 * SS], U8, isOutput=False)
    wpack_e = nc.declare_dram_parameter("wpack", [128, NWPACK], BF16, isOutput=False)
    pack_e = nc.declare_dram_parameter("pack", [128, NPACK], F32R, isOutput=False)
    biasT_e = nc.declare_dram_parameter("biasT", [128, 4096], BF16, isOutput=False)
    out_e = nc.declare_dram_parameter("out", [SS * L, 256], F32, isOutput=True)

    with ExitStack() as ctx:
        tc = ctx.enter_context(tile.TileContext(nc))

        # ---------------- pools ----------------
        cpool = ctx.enter_context(tc.tile_pool(name="const", bufs=1))
        xpool = ctx.enter_context(tc.tile_pool(name="x", bufs=4))
        qkpool = ctx.enter_context(tc.tile_pool(name="qk", bufs=2))
        vpool = ctx.enter_context(tc.tile_pool(name="v", bufs=2))
        gpool = ctx.enter_context(tc.tile_pool(name="g", bufs=2))
        epool = ctx.enter_context(tc.tile_pool(name="e", bufs=3))
        wpool = ctx.enter_context(tc.tile_pool(name="w", bufs=2))
        opool = ctx.enter_context(tc.tile_pool(name="o", bufs=2))
        ps_l = ctx.enter_context(
            tc.tile_pool(name="psl", bufs=1 if HEADS_4WAY else 2, space="PSUM"))
        ps_p = ctx.enter_context(tc.tile_pool(name="psp", bufs=2, space="PSUM"))
        ps_w = ctx.enter_context(tc.tile_pool(name="psw", bufs=1, space="PSUM"))

        # weights land in SBUF as bf16 straight off the DMA — no cast ops,
        # and the first projection only waits on this (small) load + inputs
        wp = cpool.tile([128, NWPACK], BF16, name="wp")
        nc.sync.dma_start(wp[:], wpack_e[:])
        mpack = cpool.tile([128, 2 * SS], U8, name="mpack")
        nc.sync.dma_start(mpack[:], maskT_e[:])
        cpack = cpool.tile([128, NPACK], F32R, name="cpack")
        nc.sync.dma_start(cpack[:], pack_e[:])
        biasp = cpool.tile([128, 4096], BF16, name="biasp")

        wq_sb = [wp[:, OFF_WQ + 256 * kc:OFF_WQ + 256 * (kc + 1)] for kc in range(2)]
        wk_sb = [wp[:, OFF_WK + 256 * kc:OFF_WK + 256 * (kc + 1)] for kc in range(2)]
        wg_sb = [wp[:, OFF_WG + 256 * kc:OFF_WG + 256 * (kc + 1)] for kc in range(2)]
        wv_r = [wp[:, OFF_WV + 256 * kc:OFF_WV + 256 * (kc + 1)] for kc in range(2)]
        wo_sb = [wp[:, OFF_WO + 256 * c:OFF_WO + 256 * (c + 1)] for c in range(2)]

        if USE_BO_MM:
            # bo delivered by a K=1 ones-row matmul into the out-proj group
            ones1 = cpool.tile([1, 128], BF16, name="ones1")
            nc.gpsimd.memset(ones1[:], 1.0)
            bo1 = wp[0:1, OFF_BO1:OFF_BO1 + 256]
        else:
            bo2 = cpack[:, OFF_BO2:OFF_BO2 + 512].bitcast(F32)
        ident_sb = wp[:, OFF_ID:OFF_ID + 128]

        bghalf = cpool.tile([128, 2], F32, name="bghalf")
        nc.vector.tensor_scalar_mul(
            bghalf[:], cpack[:, OFF_BG:OFF_BG + 2].bitcast(F32), 0.5)

        twos_sb = cpool.tile([128, 32], BF16, name="twos_sb")
        nc.gpsimd.memset(twos_sb[:], 2.0)

        # exp(bias), computed once; layout matches the exp tiles: e2[kc]
        # col = 256*POS[h] + q  (kc-chunk of k on partitions)
        ebb = [cpool.tile([128, 2048], BF16, name=f"ebb{kc}") for kc in range(2)]

        # HAM warmup: trip the activity monitor to 8/8 during the initial
        # DMA-wait window so the first real matmuls run at 2.4GHz.
        warm = cpool.tile([128, 512], BF16, name="warm")
        nc.gpsimd.memset(warm[:], 0.0)
        pwarm = ps_p.tile([128, 512], F32, tag="pp", name="pwarm")
        for i in range(18):
            nc.tensor.matmul(
                pwarm[:], warm[:, 0:128], warm[:],
                start=(i == 0), stop=(i == 17),
            )

        # mask -> additive offsets [128, SS] per k-chunk: mask*30 - 30
        maskadd_sb = []
        for kc in range(2):
            mf = cpool.tile([128, SS], F32, name=f"maskadd{kc}")
            nc.vector.tensor_scalar(
                mf[:], mpack[:, SS * kc:SS * (kc + 1)], -MASK_NEG, MASK_NEG,
                op0=mybir.AluOpType.mult, op1=mybir.AluOpType.add,
            )
            maskadd_sb.append(mf)

        def _attn_tail(s, expT, v_si, gtan, si):
            # ---- wavg (dense) + denominators ----
            # head h -> pw[32*(POS[h]//2), 256*(POS[h]%2)]; issue order
            # alternates col groups so streams overlap
            pw = ps_w.tile([128, 512], F32, tag="pw", name="pw")
            pd = ps_w.tile([128, 512], F32, tag="pd", name="pd")
            horder = sorted(range(8), key=lambda h: (POS[h] % 2, POS[h] // 2))
            for h in horder:
                j, c = POS[h] // 2, POS[h] % 2
                for kc in range(2):
                    nc.tensor.matmul(
                        pw[32 * j:32 * (j + 1), 256 * c:256 * (c + 1)],
                        v_si[:, 256 * kc + 32 * h:256 * kc + 32 * (h + 1)],
                        expT[kc][:, 256 * POS[h]:256 * (POS[h] + 1)],
                        start=(kc == 0), stop=(kc == 1),
                        tile_position=(0, 32 * j),
                    )
            for j in range(4):
                for kc in range(2):
                    nc.tensor.matmul(
                        pd[32 * j:32 * (j + 1), :],
                        twos_sb[:],
                        expT[kc][:, 512 * j:512 * (j + 1)],
                        start=(kc == 0), stop=(kc == 1),
                        tile_position=(0, 32 * j),
                    )

            recipb = wpool.tile([128, 512], F32, tag="recipb", name="recipb")
            nc.vector.reciprocal_approx_fast(recipb[:], pd[:])

            # t1 = (tanh + 1) * wavg_unnorm, fused straight from PSUM
            # (the sigmoid's 0.5 is folded into the 2.0-constant
            # denominator matmul); gated = t1 * 1/(2*denom)
            gtan_si = gtan[:].rearrange("p (c sq) -> p c sq", c=2)[
                :, :, 256 * si:256 * (si + 1)]
            t1 = wpool.tile([128, 512], BF16, tag="t1", name="t1")
            nc.vector.scalar_tensor_tensor(
                t1[:].rearrange("p (c q) -> p c q", c=2), gtan_si, 1.0,
                pw[:].rearrange("p (c q) -> p c q", c=2),
                op0=mybir.AluOpType.add, op1=mybir.AluOpType.mult,
            )
            gated = wpool.tile([128, 512], BF16, tag="gated", name="gated")
            if GP_GATED:
                nc.gpsimd.tensor_mul(gated[:], t1[:], recipb[:])
            else:
                nc.vector.tensor_mul(gated[:], t1[:], recipb[:])

            # ---- output projection ----
            po = ps_w.tile([128, 512], F32, tag="pd", name="po")
            for lc in range(2):
                if USE_BO_MM:
                    nc.tensor.matmul(
                        po[:, 256 * lc:256 * (lc + 1)],
                        ones1[:], bo1[:], start=True, stop=False,
                    )
                for c in range(2):
                    nc.tensor.matmul(
                        po[:, 256 * lc:256 * (lc + 1)],
                        gated[:, 256 * c + 128 * lc:256 * c + 128 * (lc + 1)],
                        wo_sb[c][:],
                        start=(not USE_BO_MM and c == 0), stop=(c == 1),
                    )
            osb = opool.tile([128, 512], F32, tag="osb", name="osb")
            if USE_BO_MM:
                # evacuate on ScalarE (VectorE is the busier engine)
                nc.scalar.activation(osb[:], po[:], AF.Copy)
            else:
                nc.vector.tensor_add(osb[:], po[:], bo2)
            nc.sync.dma_start(
                out_e[L * s:L * s + 256, :].rearrange("(lc p) o -> p lc o", lc=2),
                osb[:].rearrange("p (lc o) -> p lc o", lc=2))

        pending = None
        for sp in range(SS // 2):
            # ---- load transposed inputs: col = 512*kc + 256*si + l ----
            xq2 = xpool.tile([128, 1024], in_dt, tag="xq2", name="xq2")
            xk2 = xpool.tile([128, 1024], in_dt, tag="xk2", name="xk2")
            for si in range(2):
                s = 2 * sp + si
                # one DMA per (seq, tensor): [256,256] DRAM -> [p, kc, l]
                nc.sync.dma_start(
                    xq2[:].rearrange("p (kc sl) -> p kc sl", kc=2)[
                        :, :, 256 * si:256 * si + 256],
                    xqT_e[s].rearrange("(kc p) l -> p kc l", kc=2))
                nc.sync.dma_start(
                    xk2[:].rearrange("p (kc sl) -> p kc sl", kc=2)[
                        :, :, 256 * si:256 * si + 256],
                    xkT_e[s].rearrange("(kc p) l -> p kc l", kc=2))
            if sp == 0:
                # bias load queued AFTER the first pair's inputs so the first
                # projection matmuls aren't stuck behind it in the DMA queue
                nc.sync.dma_start(biasp[:], biasT_e[:])
                for kc in range(2):
                    for half in range(2):
                        nc.scalar.activation(
                            ebb[kc][:, 1024 * half:1024 * (half + 1)],
                            biasp[:, 2048 * kc + 1024 * half:
                                  2048 * kc + 1024 * (half + 1)],
                            AF.Exp)

            xqr, xkr = xq2[:], xk2[:]

            # ---- projections (pair-merged, N=512) ----
            qT, kT = [], []
            for m in range(2):
                pq = ps_p.tile([128, 512], F32, tag="pp", name="pq")
                for kc in range(2):
                    nc.tensor.matmul(
                        pq[:], wq_sb[kc][:, 128 * m:128 * (m + 1)],
                        xqr[:, 512 * kc:512 * (kc + 1)],
                        start=(kc == 0), stop=(kc == 1),
                    )
                qt = qkpool.tile([128, 512], BF16, tag=f"qT{m}", name=f"qT{m}")
                nc.vector.tensor_scalar_mul(qt[:], pq[:], SCALE)
                qT.append(qt)

                pk = ps_p.tile([128, 512], F32, tag="pp", name="pk")
                for kc in range(2):
                    nc.tensor.matmul(
                        pk[:], wk_sb[kc][:, 128 * m:128 * (m + 1)],
                        xkr[:, 512 * kc:512 * (kc + 1)],
                        start=(kc == 0), stop=(kc == 1),
                    )
                kt = qkpool.tile([128, 512], BF16, tag=f"kT{m}", name=f"kT{m}")
                nc.vector.tensor_copy(kt[:], pk[:])
                kT.append(kt)

            # ---- v (per seq): cols = 256*lc + (32h + d); lc = k-pos chunk ----
            v_sb = []
            for si in range(2):
                pv = ps_p.tile([128, 512], F32, tag="pp", name="pv")
                for lc in range(2):
                    for kc in range(2):
                        nc.tensor.matmul(
                            pv[:, 256 * lc:256 * (lc + 1)],
                            xkr[:, 512 * kc + 256 * si + 128 * lc:
                                512 * kc + 256 * si + 128 * (lc + 1)],
                            wv_r[kc], start=(kc == 0), stop=(kc == 1),
                        )
                vt = vpool.tile([128, 512], BF16, tag=f"v{si}", name=f"v{si}")
                if si == 0:
                    # one of the two v evacuations rides ScalarE: VectorE is
                    # the pacing engine, ScalarE has a little headroom
                    nc.scalar.activation(vt[:], pv[:], AF.Copy)
                else:
                    nc.vector.tensor_copy(vt[:], pv[:])
                v_sb.append(vt)

            # ---- gate pre-activation (dense, pair-merged): cols 512*c+256*si+q
            gtan = gpool.tile([128, 1024], BF16, tag="gtan", name="gtan")
            for c in range(2):
                pg = ps_p.tile([128, 512], F32, tag="pp", name="pg")
                for kc in range(2):
                    nc.tensor.matmul(
                        pg[:], wg_sb[kc][:, 128 * c:128 * (c + 1)],
                        xqr[:, 512 * kc:512 * (kc + 1)],
                        start=(kc == 0), stop=(kc == 1),
                    )
                nc.scalar.activation(
                    gtan[:, 512 * c:512 * (c + 1)], pg[:],
                    AF.Tanh, bias=bghalf[:, c:c + 1], scale=0.5,
                )

            for si in range(2):
                s = 2 * sp + si
                # ---- logits + exp + bias-multiply ----
                # (the attention tail for this sequence is deferred to the
                # next iteration — see _attn_tail below — so the PE fills
                # the exp-ACT latency with the previous sequence's work)
                expT = []
                for kc in range(2):
                    e2 = epool.tile([128, H * L], BF16, tag=f"exp{kc}", name=f"exp{kc}")
                    pe_bias_t = HEADS_4WAY and PE_BIAS and (2 * s + kc) % PE_BIAS == 0
                    if not INPLACE_EB and not pe_bias_t:
                        e2w = epool.tile([128, H * L], BF16, tag=f"expw{kc}",
                                         name=f"expw{kc}")
                    if HEADS_4WAY:
                        # one 4-bank tile; quads (0,2,4,6)/(1,3,5,7): distinct
                        # banks AND distinct row groups within each quad
                        pe_bias = pe_bias_t
                        pl = ps_l.tile([128, 2048], F32, tag="pl", name="pl")
                        if pe_bias:
                            # bias lands in PSUM via ident broadcast; heads
                            # then accumulate on top
                            for b2 in range(4):
                                nc.tensor.matmul(
                                    pl[:, 512 * b2:512 * (b2 + 1)],
                                    ident_sb,
                                    biasp[:, 2048 * kc + 512 * b2:
                                          2048 * kc + 512 * (b2 + 1)],
                                    start=True, stop=False,
                                    skip_group_check=True,
                                )
                        for b in (0, 2, 4, 6, 1, 3, 5, 7):
                            h = HEAD_AT[b]
                            m, r = h // 4, 32 * (h % 4)
                            nc.tensor.matmul(
                                pl[:, 256 * b:256 * (b + 1)],
                                kT[m][r:r + 32, 256 * si + 128 * kc:
                                      256 * si + 128 * (kc + 1)],
                                qT[m][r:r + 32, 256 * si:256 * (si + 1)],
                                start=not pe_bias, stop=True,
                                tile_position=(r, 0), skip_group_check=True,
                            )
                        # evacuate per half: releases PSUM banks A,B after the
                        # first ACT so the next group's quads start earlier
                        dst = e2 if pe_bias or INPLACE_EB else e2w
                        for half in range(2):
                            sl = slice(1024 * half, 1024 * (half + 1))
                            nc.scalar.activation(
                                e2[:, sl], pl[:, sl],
                                AF.Exp, bias=maskadd_sb[kc][:, s:s + 1])
                            if pe_bias:
                                pass
                            elif EB_GP and (2 * s + kc) % EB_GP == 0:
                                nc.gpsimd.tensor_mul(dst[:, sl], e2[:, sl],
                                                     ebb[kc][:, sl])
                            else:
                                nc.vector.tensor_mul(dst[:, sl], e2[:, sl],
                                                     ebb[kc][:, sl])
                        expT.append(e2 if (pe_bias or INPLACE_EB) else e2w)
                        continue
                    else:
                        for half in range(2):
                            pl = ps_l.tile([128, 1024], F32, tag="pl", name="pl")
                            for hh in range(4):
                                h = HEAD_AT[4 * half + hh]
                                m, r = h // 4, 32 * (h % 4)
                                nc.tensor.matmul(
                                    pl[:, 256 * hh:256 * (hh + 1)],
                                    kT[m][r:r + 32, 256 * si + 128 * kc:
                                          256 * si + 128 * (kc + 1)],
                                    qT[m][r:r + 32, 256 * si:256 * (si + 1)],
                                    start=True, stop=True,
                                    tile_position=(r, 0), skip_group_check=True,
                                )
                            nc.scalar.activation(
                                e2[:, 1024 * half:1024 * (half + 1)], pl[:],
                                AF.Exp, bias=maskadd_sb[kc][:, s:s + 1])
                            dst = e2 if INPLACE_EB else e2w
                            nc.vector.tensor_mul(
                                dst[:, 1024 * half:1024 * (half + 1)],
                                e2[:, 1024 * half:1024 * (half + 1)],
                                ebb[kc][:, 1024 * half:1024 * (half + 1)])
                    expT.append(e2 if INPLACE_EB else e2w)

                if pending is not None:
                    _attn_tail(**pending)
                pending = dict(s=s, expT=expT, v_si=v_sb[si], gtan=gtan, si=si)

        if pending is not None:
            _attn_tail(**pending)

    nc.finalize()
    return nc


def _host_prep(q_data, k_data, bias, k_mask, Wq, Wk, Wv, Wg, bg, Wo, bo):
    """Pure layout transforms (transpose / permute / pad); no arithmetic."""
    q_data = np.ascontiguousarray(np.asarray(q_data, dtype=np.float32))
    k_data = np.ascontiguousarray(np.asarray(k_data, dtype=np.float32))
    bias = np.asarray(bias, dtype=np.float32)
    k_mask = np.asarray(k_mask)

    xqT = np.ascontiguousarray(q_data[0].transpose(0, 2, 1))   # [S, C, L]
    xkT = np.ascontiguousarray(k_data[0].transpose(0, 2, 1))
    if USE_BF16_PROJ:
        import ml_dtypes
        xqT = xqT.astype(ml_dtypes.bfloat16)
        xkT = xkT.astype(ml_dtypes.bfloat16)
    biasT_h = bias[0].transpose(2, 0, 1)          # [k, h, q]
    biasT = np.zeros((L, H * L), np.float32)
    for h in range(H):
        biasT[:, 256 * POS[h]:256 * (POS[h] + 1)] = biasT_h[:, h, :]
    maskT_all = np.ascontiguousarray(k_mask[0].astype(np.uint8).T)  # [L, S]

    perm = [np.array(PERM_HD[c]) for c in range(2)]
    import ml_dtypes

    wpack = np.zeros((128, NWPACK), np.float32)
    pack = np.zeros((128, NPACK), np.float32)
    Wq_ = np.asarray(Wq, np.float32); Wk_ = np.asarray(Wk, np.float32)
    Wv_ = np.asarray(Wv, np.float32); Wg_ = np.asarray(Wg, np.float32)
    Wo_ = np.asarray(Wo, np.float32)
    bo_ = np.asarray(bo, np.float32)
    bg_ = np.asarray(bg, np.float32)
    for kc in range(2):
        wpack[:, OFF_WQ + 256 * kc:OFF_WQ + 256 * (kc + 1)] = Wq_[128 * kc:128 * (kc + 1)]
        wpack[:, OFF_WK + 256 * kc:OFF_WK + 256 * (kc + 1)] = Wk_[128 * kc:128 * (kc + 1)]
        wpack[:, OFF_WV + 256 * kc:OFF_WV + 256 * (kc + 1)] = Wv_[128 * kc:128 * (kc + 1)]
        # Wg columns permuted: packed col 128c+p = Wg[:, PERM_HD[c][p]]
        for c in range(2):
            wpack[:, OFF_WG + 256 * kc + 128 * c:OFF_WG + 256 * kc + 128 * (c + 1)] = \
                Wg_[128 * kc:128 * (kc + 1)][:, perm[c]]
        pack[:, OFF_BO2 + 256 * kc:OFF_BO2 + 256 * (kc + 1)] = bo_[None, :]
    # Wo rows permuted: packed row p of block c = Wo[PERM_HD[c][p]]
    for c in range(2):
        wpack[:, OFF_WO + 256 * c:OFF_WO + 256 * (c + 1)] = Wo_[perm[c]][:, :]
    wpack[0, OFF_BO1:OFF_BO1 + 256] = bo_
    wpack[:, OFF_ID:OFF_ID + 128] = np.eye(128, dtype=np.float32)
    for c in range(2):
        pack[:, OFF_BG + c] = bg_[perm[c]]

    biasT_p = np.concatenate([biasT[0:128], biasT[128:256]], axis=1)  # [128, 4096]
    biasT_p = biasT_p.astype(ml_dtypes.bfloat16)
    common = dict(pack=pack, biasT=np.ascontiguousarray(biasT_p),
                  wpack=np.ascontiguousarray(wpack.astype(ml_dtypes.bfloat16)))
    in_maps = []
    for i in range(NCORES):
        m = dict(common)
        m["xqT"] = np.ascontiguousarray(xqT[SS * i:SS * (i + 1)])
        m["xkT"] = np.ascontiguousarray(xkT[SS * i:SS * (i + 1)])
        md = np.zeros((128, 2 * SS), np.uint8)
        mt = maskT_all[:, SS * i:SS * (i + 1)]
        md[:, 0:SS] = mt[0:128]; md[:, SS:2 * SS] = mt[128:256]
        m["maskT"] = md
        in_maps.append(m)
    return in_maps


def kernel(q_data, k_data, bias, k_mask, Wq, Wk, Wv, Wg, bg, Wo, bo):
    in_maps = _host_prep(q_data, k_data, bias, k_mask, Wq, Wk, Wv, Wg, bg, Wo, bo)
    if "nc" not in _CACHE:
        _CACHE["nc"] = _build_nc()
    trace = bool(int(os.environ.get("KERNEL_TRACE", "0")))
    res = run_bass_kernel_spmd(
        _CACHE["nc"], in_maps, core_ids=list(range(NCORES)), trace=trace,
    )
    _CACHE["last_result"] = res
    out = np.concatenate([res.results[i]["out"] for i in range(NCORES)], axis=0)
    return out.reshape(1, S, L, 256)


# revision 46
# speedup vs baseline: 1.0032x; 1.0032x over previous
"""Trainium2 Bass kernel: gated MSA row attention (AlphaFold-style).

Shapes: q_data/k_data [1,128,256,256], bias [1,8,256,256], k_mask [1,128,256].
Sharding: data-parallel over the 128 sequences -> 16 per core on 8 cores.

v3 design notes (vs the v2 baseline, 179us -> ~152us):
- Bias delivery via exp-factoring: weights = exp(logits+mask) * exp(bias).
  exp(bias) is computed ONCE per core (4 ACTs); the per-sequence ident
  bias-broadcast matmuls (128 insts, ~27us of PE time) are gone, replaced
  by a bf16 VectorE multiply on the exp tiles (2x_1P DVE mode; NOT
  in-place — out==in0 drops the op to 1x on HW).
- Head block order POS=[0,2,4,6,1,3,5,7]: the 8 head matmuls of one
  (si,kc) go into ONE 4-bank PSUM tile as two quads, each quad hitting 4
  distinct row groups AND 4 distinct banks -> true 4-way tile
  concurrency (identity order put 2 concurrent streams on one PSUM bank
  -> hardware error; v2's order was bank-clean but only 2-way).
  Wo rows / Wg cols / bg are permuted on the host (pure layout) so the
  gate/out-proj algebra is unchanged.
- Software pipelining: each sequence's tail (wavg/denom/gate/out-proj)
  is issued during the NEXT sequence's logits phase, so the PE fills the
  exp-ACT latency instead of head-of-line blocking (engine queues are
  program-order FIFOs).
- Everything ships bf16 from the host (inputs, weights, bias): no
  on-device casts, half the DMA bytes, and bf16 matmuls stream 2x faster
  than fp32 (fp32_mode=HIGH measured ~1.3ns/col).
- The exp evacuation is split per 2-bank half (earlier PSUM release);
  input DMAs are merged to one descriptor per (seq, tensor).
- bo is added by a K=1 ones-row matmul into the out-proj PSUM group; the
  PSUM->SBUF output evacuation and one v-evacuation per pair run on
  ScalarE (AF.Copy) to offload VectorE, which is the pacing engine.
- GpSimd is left idle on purpose: its elementwise ops run at ~0.4x DVE
  AND poison concurrent DVE throughput via SBUF port contention
  (measured: gated-mul on GpSimd cost +16us end-to-end).
"""

import os
import sys
import numpy as np
from contextlib import ExitStack

sys.path.insert(0, "/opt/trn_rl_repo")

import concourse.bass as bass
import concourse.bacc as bacc
import concourse.mybir as mybir
from concourse import tile
from concourse.bass_utils import run_bass_kernel_spmd

NCORES = 8
S = 128
SS = S // NCORES          # 16 sequences per core
L = 256                   # residues (q and k length)
C = 256                   # channels
H = 8                     # heads
DK = 32                   # head dim
SCALE = 1.0 / np.sqrt(DK)
MASK_NEG = -30.0          # additive logit offset for masked keys

F32 = mybir.dt.float32
F32R = mybir.dt.float32r
BF16 = mybir.dt.bfloat16
U8 = mybir.dt.uint8
AF = mybir.ActivationFunctionType

# bf16 weight pack (DMA'd first; weights usable with no on-device casts)
OFF_WQ = 0
OFF_WK = OFF_WQ + 512
OFF_WV = OFF_WK + 512
OFF_WG = OFF_WV + 512
OFF_WO = OFF_WG + 512
OFF_BO1 = OFF_WO + 512     # bo on row 0, bf16
OFF_ID = OFF_BO1 + 256     # 128x128 identity, bf16 (PE bias broadcast)
NWPACK = OFF_ID + 128
# small f32 pack (bo2 broadcast for the no-BOMM path, bg)
OFF_BO2 = 0
OFF_BG = OFF_BO2 + 512
NPACK = OFF_BG + 2

# feature toggles for HW bisection (sim passes with all on)
USE_BF16_PROJ = bool(int(os.environ.get("K_BF16", "1")))   # bf16 inputs + weights
USE_BO_MM = bool(int(os.environ.get("K_BOMM", "1")))       # bo via K=1 matmul
HEADS_4WAY = bool(int(os.environ.get("K_H4", "1")))        # identity head order
GP_GATED = bool(int(os.environ.get("K_GPG", "0")))         # gated mul on gpsimd
INPLACE_EB = bool(int(os.environ.get("K_INPL", "0")))      # e2 *= ebb in place
EB_GP = int(os.environ.get("K_EBGP", "0"))                 # every Nth eb-mult on gpsimd (0=off)
# every Nth (s,kc) tile gets bias via PE ident-broadcast matmuls instead of
# the VectorE exp(bias) multiply (0=off): VectorE is the pacer, PE has slack
PE_BIAS = int(os.environ.get("K_PEB", "0"))
# split the exp ACT + eb multiply per 2-bank half: costs ScalarE ~143ns init
# per extra ACT and the heads need ALL 4 banks anyway -> default merged
SPLIT_ACT = bool(int(os.environ.get("K_SPLITACT", "0")))

# head h -> exp/logits block POS[h]; HEAD_AT = inverse.
# 4-way mode: logits go to ONE 4-bank PSUM tile [128,2048]; blocks are
# issued as quads (0,2,4,6) then (1,3,5,7) so each concurrent stream has
# its own bank (2 streams draining one PSUM bank is a HW error) AND its
# own PE row group (r = 32*(HEAD_AT[b]%4) distinct within a quad).
# v2 order ([h0,h4|h1,h5|h2,h6|h3,h7]) pairs banks cleanly but only gets
# 2-way row concurrency.
if HEADS_4WAY:
    POS = [0, 2, 4, 6, 1, 3, 5, 7]
else:
    POS = [2 * (h % 4) + (h // 4) for h in range(8)]
HEAD_AT = [0] * 8
for _h in range(8):
    HEAD_AT[POS[_h]] = _h

# partition p of the gated/wavg col-block c holds hd = 32*HEAD_AT[2*(p//32)+c] + p%32
PERM_HD = [[32 * HEAD_AT[2 * (p // 32) + c] + (p % 32) for p in range(128)]
           for c in range(2)]

_CACHE = {}


def _build_nc():
    nc = bacc.Bacc()

    assert USE_BF16_PROJ, "f32r path removed: weights ship as bf16"
    in_dt = BF16
    xqT_e = nc.declare_dram_parameter("xqT", [SS, C, L], in_dt, isOutput=False)
    xkT_e = nc.declare_dram_parameter("xkT", [SS, C, L], in_dt, isOutput=False)
    maskT_e = nc.declare_dram_parameter("maskT", [128, 2 * SS], U8, isOutput=False)
    wpack_e = nc.declare_dram_parameter("wpack", [128, NWPACK], BF16, isOutput=False)
    pack_e = nc.declare_dram_parameter("pack", [128, NPACK], F32R, isOutput=False)
    biasT_e = nc.declare_dram_parameter("biasT", [128, 4096], BF16, isOutput=False)
    out_e = nc.declare_dram_parameter("out", [SS * L, 256], F32, isOutput=True)

    with ExitStack() as ctx:
        tc = ctx.enter_context(tile.TileContext(nc))

        # ---------------- pools ----------------
        cpool = ctx.enter_context(tc.tile_pool(name="const", bufs=1))
        xpool = ctx.enter_context(tc.tile_pool(name="x", bufs=4))
        qkpool = ctx.enter_context(tc.tile_pool(name="qk", bufs=2))
        vpool = ctx.enter_context(tc.tile_pool(name="v", bufs=2))
        gpool = ctx.enter_context(tc.tile_pool(name="g", bufs=2))
        epool = ctx.enter_context(tc.tile_pool(name="e", bufs=3))
        wpool = ctx.enter_context(tc.tile_pool(name="w", bufs=2))
        opool = ctx.enter_context(tc.tile_pool(name="o", bufs=2))
        ps_l = ctx.enter_context(
            tc.tile_pool(name="psl", bufs=1 if HEADS_4WAY else 2, space="PSUM"))
        ps_p = ctx.enter_context(tc.tile_pool(name="psp", bufs=2, space="PSUM"))
        ps_w = ctx.enter_context(tc.tile_pool(name="psw", bufs=1, space="PSUM"))

        # weights land in SBUF as bf16 straight off the DMA — no cast ops,
        # and the first projection only waits on this (small) load + inputs
        wp = cpool.tile([128, NWPACK], BF16, name="wp")
        nc.sync.dma_start(wp[:], wpack_e[:])
        mpack = cpool.tile([128, 2 * SS], U8, name="mpack")
        nc.sync.dma_start(mpack[:], maskT_e[:])
        cpack = cpool.tile([128, NPACK], F32R, name="cpack")
        nc.sync.dma_start(cpack[:], pack_e[:])
        biasp = cpool.tile([128, 4096], BF16, name="biasp")

        wq_sb = [wp[:, OFF_WQ + 256 * kc:OFF_WQ + 256 * (kc + 1)] for kc in range(2)]
        wk_sb = [wp[:, OFF_WK + 256 * kc:OFF_WK + 256 * (kc + 1)] for kc in range(2)]
        wg_sb = [wp[:, OFF_WG + 256 * kc:OFF_WG + 256 * (kc + 1)] for kc in range(2)]
        wv_r = [wp[:, OFF_WV + 256 * kc:OFF_WV + 256 * (kc + 1)] for kc in range(2)]
        wo_sb = [wp[:, OFF_WO + 256 * c:OFF_WO + 256 * (c + 1)] for c in range(2)]

        if USE_BO_MM:
            # bo delivered by a K=1 ones-row matmul into the out-proj group
            ones1 = cpool.tile([1, 128], BF16, name="ones1")
            nc.gpsimd.memset(ones1[:], 1.0)
            bo1 = wp[0:1, OFF_BO1:OFF_BO1 + 256]
        else:
            bo2 = cpack[:, OFF_BO2:OFF_BO2 + 512].bitcast(F32)
        ident_sb = wp[:, OFF_ID:OFF_ID + 128]

        bghalf = cpool.tile([128, 2], F32, name="bghalf")
        nc.vector.tensor_scalar_mul(
            bghalf[:], cpack[:, OFF_BG:OFF_BG + 2].bitcast(F32), 0.5)

        twos_sb = cpool.tile([128, 32], BF16, name="twos_sb")
        nc.gpsimd.memset(twos_sb[:], 2.0)

        # exp(bias), computed once; layout matches the exp tiles: e2[kc]
        # col = 256*POS[h] + q  (kc-chunk of k on partitions)
        ebb = [cpool.tile([128, 2048], BF16, name=f"ebb{kc}") for kc in range(2)]

        # HAM warmup: trip the activity monitor to 8/8 during the initial
        # DMA-wait window so the first real matmuls run at 2.4GHz.
        warm = cpool.tile([128, 512], BF16, name="warm")
        nc.gpsimd.memset(warm[:], 0.0)
        pwarm = ps_p.tile([128, 512], F32, tag="pp", name="pwarm")
        for i in range(18):
            nc.tensor.matmul(
                pwarm[:], warm[:, 0:128], warm[:],
                start=(i == 0), stop=(i == 17),
            )

        # mask -> additive offsets [128, SS] per k-chunk: mask*30 - 30
        maskadd_sb = []
        for kc in range(2):
            mf = cpool.tile([128, SS], F32, name=f"maskadd{kc}")
            nc.vector.tensor_scalar(
                mf[:], mpack[:, SS * kc:SS * (kc + 1)], -MASK_NEG, MASK_NEG,
                op0=mybir.AluOpType.mult, op1=mybir.AluOpType.add,
            )
            maskadd_sb.append(mf)

        def _attn_tail(s, expT, v_si, gtan, si):
            # ---- wavg (dense) + denominators ----
            # head h -> pw[32*(POS[h]//2), 256*(POS[h]%2)]; issue order
            # alternates col groups so streams overlap
            pw = ps_w.tile([128, 512], F32, tag="pw", name="pw")
            pd = ps_w.tile([128, 512], F32, tag="pd", name="pd")
            horder = sorted(range(8), key=lambda h: (POS[h] % 2, POS[h] // 2))
            for h in horder:
                j, c = POS[h] // 2, POS[h] % 2
                for kc in range(2):
                    nc.tensor.matmul(
                        pw[32 * j:32 * (j + 1), 256 * c:256 * (c + 1)],
                        v_si[:, 256 * kc + 32 * h:256 * kc + 32 * (h + 1)],
                        expT[kc][:, 256 * POS[h]:256 * (POS[h] + 1)],
                        start=(kc == 0), stop=(kc == 1),
                        tile_position=(0, 32 * j),
                    )
            for j in range(4):
                for kc in range(2):
                    nc.tensor.matmul(
                        pd[32 * j:32 * (j + 1), :],
                        twos_sb[:],
                        expT[kc][:, 512 * j:512 * (j + 1)],
                        start=(kc == 0), stop=(kc == 1),
                        tile_position=(0, 32 * j),
                    )

            recipb = wpool.tile([128, 512], F32, tag="recipb", name="recipb")
            nc.vector.reciprocal_approx_fast(recipb[:], pd[:])

            # t1 = (tanh + 1) * wavg_unnorm, fused straight from PSUM
            # (the sigmoid's 0.5 is folded into the 2.0-constant
            # denominator matmul); gated = t1 * 1/(2*denom)
            gtan_si = gtan[:].rearrange("p (c sq) -> p c sq", c=2)[
                :, :, 256 * si:256 * (si + 1)]
            t1 = wpool.tile([128, 512], BF16, tag="t1", name="t1")
            nc.vector.scalar_tensor_tensor(
                t1[:].rearrange("p (c q) -> p c q", c=2), gtan_si, 1.0,
                pw[:].rearrange("p (c q) -> p c q", c=2),
                op0=mybir.AluOpType.add, op1=mybir.AluOpType.mult,
            )
            gated = wpool.tile([128, 512], BF16, tag="gated", name="gated")
            if GP_GATED:
                nc.gpsimd.tensor_mul(gated[:], t1[:], recipb[:])
            else:
                nc.vector.tensor_mul(gated[:], t1[:], recipb[:])

            # ---- output projection ----
            po = ps_w.tile([128, 512], F32, tag="pd", name="po")
            for lc in range(2):
                if USE_BO_MM:
                    nc.tensor.matmul(
                        po[:, 256 * lc:256 * (lc + 1)],
                        ones1[:], bo1[:], start=True, stop=False,
                    )
                for c in range(2):
                    nc.tensor.matmul(
                        po[:, 256 * lc:256 * (lc + 1)],
                        gated[:, 256 * c + 128 * lc:256 * c + 128 * (lc + 1)],
                        wo_sb[c][:],
                        start=(not USE_BO_MM and c == 0), stop=(c == 1),
                    )
            osb = opool.tile([128, 512], F32, tag="osb", name="osb")
            if USE_BO_MM:
                # evacuate on ScalarE (VectorE is the busier engine)
                nc.scalar.activation(osb[:], po[:], AF.Copy)
            else:
                nc.vector.tensor_add(osb[:], po[:], bo2)
            nc.sync.dma_start(
                out_e[L * s:L * s + 256, :].rearrange("(lc p) o -> p lc o", lc=2),
                osb[:].rearrange("p (lc o) -> p lc o", lc=2))

        pending = None
        for sp in range(SS // 2):
            # ---- load transposed inputs: col = 512*kc + 256*si + l ----
            xq2 = xpool.tile([128, 1024], in_dt, tag="xq2", name="xq2")
            xk2 = xpool.tile([128, 1024], in_dt, tag="xk2", name="xk2")
            for si in range(2):
                s = 2 * sp + si
                # one DMA per (seq, tensor): [256,256] DRAM -> [p, kc, l]
                nc.sync.dma_start(
                    xq2[:].rearrange("p (kc sl) -> p kc sl", kc=2)[
                        :, :, 256 * si:256 * si + 256],
                    xqT_e[s].rearrange("(kc p) l -> p kc l", kc=2))
                nc.sync.dma_start(
                    xk2[:].rearrange("p (kc sl) -> p kc sl", kc=2)[
                        :, :, 256 * si:256 * si + 256],
                    xkT_e[s].rearrange("(kc p) l -> p kc l", kc=2))
            if sp == 0:
                # bias load queued AFTER the first pair's inputs so the first
                # projection matmuls aren't stuck behind it in the DMA queue
                nc.sync.dma_start(biasp[:], biasT_e[:])
                for kc in range(2):
                    for half in range(2):
                        nc.scalar.activation(
                            ebb[kc][:, 1024 * half:1024 * (half + 1)],
                            biasp[:, 2048 * kc + 1024 * half:
                                  2048 * kc + 1024 * (half + 1)],
                            AF.Exp)

            xqr, xkr = xq2[:], xk2[:]

            # ---- projections (pair-merged, N=512) ----
            qT, kT = [], []
            for m in range(2):
                pq = ps_p.tile([128, 512], F32, tag="pp", name="pq")
                for kc in range(2):
                    nc.tensor.matmul(
                        pq[:], wq_sb[kc][:, 128 * m:128 * (m + 1)],
                        xqr[:, 512 * kc:512 * (kc + 1)],
                        start=(kc == 0), stop=(kc == 1),
                    )
                qt = qkpool.tile([128, 512], BF16, tag=f"qT{m}", name=f"qT{m}")
                nc.vector.tensor_scalar_mul(qt[:], pq[:], SCALE)
                qT.append(qt)

                pk = ps_p.tile([128, 512], F32, tag="pp", name="pk")
                for kc in range(2):
                    nc.tensor.matmul(
                        pk[:], wk_sb[kc][:, 128 * m:128 * (m + 1)],
                        xkr[:, 512 * kc:512 * (kc + 1)],
                        start=(kc == 0), stop=(kc == 1),
                    )
                kt = qkpool.tile([128, 512], BF16, tag=f"kT{m}", name=f"kT{m}")
                nc.vector.tensor_copy(kt[:], pk[:])
                kT.append(kt)

            # ---- v (per seq): cols = 256*lc + (32h + d); lc = k-pos chunk ----
            v_sb = []
            for si in range(2):
                pv = ps_p.tile([128, 512], F32, tag="pp", name="pv")
                for lc in range(2):
                    for kc in range(2):
                        nc.tensor.matmul(
                            pv[:, 256 * lc:256 * (lc + 1)],
                            xkr[:, 512 * kc + 256 * si + 128 * lc:
                                512 * kc + 256 * si + 128 * (lc + 1)],
                            wv_r[kc], start=(kc == 0), stop=(kc == 1),
                        )
                vt = vpool.tile([128, 512], BF16, tag=f"v{si}", name=f"v{si}")
                if si == 0:
                    # one of the two v evacuations rides ScalarE: VectorE is
                    # the pacing engine, ScalarE has a little headroom
                    nc.scalar.activation(vt[:], pv[:], AF.Copy)
                else:
                    nc.vector.tensor_copy(vt[:], pv[:])
                v_sb.append(vt)

            # ---- gate pre-activation (dense, pair-merged): cols 512*c+256*si+q
            gtan = gpool.tile([128, 1024], BF16, tag="gtan", name="gtan")
            for c in range(2):
                pg = ps_p.tile([128, 512], F32, tag="pp", name="pg")
                for kc in range(2):
                    nc.tensor.matmul(
                        pg[:], wg_sb[kc][:, 128 * c:128 * (c + 1)],
                        xqr[:, 512 * kc:512 * (kc + 1)],
                        start=(kc == 0), stop=(kc == 1),
                    )
                nc.scalar.activation(
                    gtan[:, 512 * c:512 * (c + 1)], pg[:],
                    AF.Tanh, bias=bghalf[:, c:c + 1], scale=0.5,
                )

            for si in range(2):
                s = 2 * sp + si
                # ---- logits + exp + bias-multiply ----
                # (the attention tail for this sequence is deferred to the
                # next iteration — see _attn_tail below — so the PE fills
                # the exp-ACT latency with the previous sequence's work)
                expT = []
                for kc in range(2):
                    e2 = epool.tile([128, H * L], BF16, tag=f"exp{kc}", name=f"exp{kc}")
                    pe_bias_t = HEADS_4WAY and PE_BIAS and (2 * s + kc) % PE_BIAS == 0
                    if not INPLACE_EB and not pe_bias_t:
                        e2w = epool.tile([128, H * L], BF16, tag=f"expw{kc}",
                                         name=f"expw{kc}")
                    if HEADS_4WAY:
                        # one 4-bank tile; quads (0,2,4,6)/(1,3,5,7): distinct
                        # banks AND distinct row groups within each quad
                        pe_bias = pe_bias_t
                        pl = ps_l.tile([128, 2048], F32, tag="pl", name="pl")
                        if pe_bias:
                            # bias lands in PSUM via ident broadcast; heads
                            # then accumulate on top
                            for b2 in range(4):
                                nc.tensor.matmul(
                                    pl[:, 512 * b2:512 * (b2 + 1)],
                                    ident_sb,
                                    biasp[:, 2048 * kc + 512 * b2:
                                          2048 * kc + 512 * (b2 + 1)],
                                    start=True, stop=False,
                                    skip_group_check=True,
                                )
                        for b in (0, 2, 4, 6, 1, 3, 5, 7):
                            h = HEAD_AT[b]
                            m, r = h // 4, 32 * (h % 4)
                            nc.tensor.matmul(
                                pl[:, 256 * b:256 * (b + 1)],
                                kT[m][r:r + 32, 256 * si + 128 * kc:
                                      256 * si + 128 * (kc + 1)],
                                qT[m][r:r + 32, 256 * si:256 * (si + 1)],
                                start=not pe_bias, stop=True,
                                tile_position=(r, 0), skip_group_check=True,
                            )
                        dst = e2 if pe_bias or INPLACE_EB else e2w
                        nhalf = 2 if SPLIT_ACT else 1
                        for half in range(nhalf):
                            w = 2048 // nhalf
                            sl = slice(w * half, w * (half + 1))
                            nc.scalar.activation(
                                e2[:, sl], pl[:, sl],
                                AF.Exp, bias=maskadd_sb[kc][:, s:s + 1])
                            if pe_bias:
                                pass
                            elif EB_GP and (2 * s + kc) % EB_GP == 0:
                                nc.gpsimd.tensor_mul(dst[:, sl], e2[:, sl],
                                                     ebb[kc][:, sl])
                            else:
                                nc.vector.tensor_mul(dst[:, sl], e2[:, sl],
                                                     ebb[kc][:, sl])
                        expT.append(e2 if (pe_bias or INPLACE_EB) else e2w)
                        continue
                    else:
                        for half in range(2):
                            pl = ps_l.tile([128, 1024], F32, tag="pl", name="pl")
                            for hh in range(4):
                                h = HEAD_AT[4 * half + hh]
                                m, r = h // 4, 32 * (h % 4)
                                nc.tensor.matmul(
                                    pl[:, 256 * hh:256 * (hh + 1)],
                                    kT[m][r:r + 32, 256 * si + 128 * kc:
                                          256 * si + 128 * (kc + 1)],
                                    qT[m][r:r + 32, 256 * si:256 * (si + 1)],
                                    start=True, stop=True,
                                    tile_position=(r, 0), skip_group_check=True,
                                )
                            nc.scalar.activation(
                                e2[:, 1024 * half:1024 * (half + 1)], pl[:],
                                AF.Exp, bias=maskadd_sb[kc][:, s:s + 1])
                            dst = e2 if INPLACE_EB else e2w
                            nc.vector.tensor_mul(
                                dst[:, 1024 * half:1024 * (half + 1)],
                                e2[:, 1024 * half:1024 * (half + 1)],
                                ebb[kc][:, 1024 * half:1024 * (half + 1)])
                    expT.append(e2 if INPLACE_EB else e2w)

                if pending is not None:
                    _attn_tail(**pending)
                pending = dict(s=s, expT=expT, v_si=v_sb[si], gtan=gtan, si=si)

        if pending is not None:
            _attn_tail(**pending)

    nc.finalize()
    return nc


def _host_prep(q_data, k_data, bias, k_mask, Wq, Wk, Wv, Wg, bg, Wo, bo):
    """Pure layout transforms (transpose / permute / pad); no arithmetic."""
    q_data = np.ascontiguousarray(np.asarray(q_data, dtype=np.float32))
    k_data = np.ascontiguousarray(np.asarray(k_data, dtype=np.float32))
    bias = np.asarray(bias, dtype=np.float32)
    k_mask = np.asarray(k_mask)

    xqT = np.ascontiguousarray(q_data[0].transpose(0, 2, 1))   # [S, C, L]
    xkT = np.ascontiguousarray(k_data[0].transpose(0, 2, 1))
    if USE_BF16_PROJ:
        import ml_dtypes
        xqT = xqT.astype(ml_dtypes.bfloat16)
        xkT = xkT.astype(ml_dtypes.bfloat16)
    biasT_h = bias[0].transpose(2, 0, 1)          # [k, h, q]
    biasT = np.zeros((L, H * L), np.float32)
    for h in range(H):
        biasT[:, 256 * POS[h]:256 * (POS[h] + 1)] = biasT_h[:, h, :]
    maskT_all = np.ascontiguousarray(k_mask[0].astype(np.uint8).T)  # [L, S]

    perm = [np.array(PERM_HD[c]) for c in range(2)]
    import ml_dtypes

    wpack = np.zeros((128, NWPACK), np.float32)
    pack = np.zeros((128, NPACK), np.float32)
    Wq_ = np.asarray(Wq, np.float32); Wk_ = np.asarray(Wk, np.float32)
    Wv_ = np.asarray(Wv, np.float32); Wg_ = np.asarray(Wg, np.float32)
    Wo_ = np.asarray(Wo, np.float32)
    bo_ = np.asarray(bo, np.float32)
    bg_ = np.asarray(bg, np.float32)
    for kc in range(2):
        wpack[:, OFF_WQ + 256 * kc:OFF_WQ + 256 * (kc + 1)] = Wq_[128 * kc:128 * (kc + 1)]
        wpack[:, OFF_WK + 256 * kc:OFF_WK + 256 * (kc + 1)] = Wk_[128 * kc:128 * (kc + 1)]
        wpack[:, OFF_WV + 256 * kc:OFF_WV + 256 * (kc + 1)] = Wv_[128 * kc:128 * (kc + 1)]
        # Wg columns permuted: packed col 128c+p = Wg[:, PERM_HD[c][p]]
        for c in range(2):
            wpack[:, OFF_WG + 256 * kc + 128 * c:OFF_WG + 256 * kc + 128 * (c + 1)] = \
                Wg_[128 * kc:128 * (kc + 1)][:, perm[c]]
        pack[:, OFF_BO2 + 256 * kc:OFF_BO2 + 256 * (kc + 1)] = bo_[None, :]
    # Wo rows permuted: packed row p of block c = Wo[PERM_HD[c][p]]
    for c in range(2):
        wpack[:, OFF_WO + 256 * c:OFF_WO + 256 * (c + 1)] = Wo_[perm[c]][:, :]
    wpack[0, OFF_BO1:OFF_BO1 + 256] = bo_
    wpack[:, OFF_ID:OFF_ID + 128] = np.eye(128, dtype=np.float32)
    for c in range(2):
        pack[:, OFF_BG + c] = bg_[perm[c]]

    biasT_p = np.concatenate([biasT[0:128], biasT[128:256]], axis=1)  # [128, 4096]
    biasT_p = biasT_p.astype(ml_dtypes.bfloat16)
    common = dict(pack=pack, biasT=np.ascontiguousarray(biasT_p),
                  wpack=np.ascontiguousarray(wpack.astype(ml_dtypes.bfloat16)))
    in_maps = []
    for i in range(NCORES):
        m = dict(common)
        m["xqT"] = np.ascontiguousarray(xqT[SS * i:SS * (i + 1)])
        m["xkT"] = np.ascontiguousarray(xkT[SS * i:SS * (i + 1)])
        md = np.zeros((128, 2 * SS), np.uint8)
        mt = maskT_all[:, SS * i:SS * (i + 1)]
        md[:, 0:SS] = mt[0:128]; md[:, SS:2 * SS] = mt[128:256]
        m["maskT"] = md
        in_maps.append(m)
    return in_maps


def kernel(q_data, k_data, bias, k_mask, Wq, Wk, Wv, Wg, bg, Wo, bo):
    in_maps = _host_prep(q_data, k_data, bias, k_mask, Wq, Wk, Wv, Wg, bg, Wo, bo)
    if "nc" not in _CACHE:
        _CACHE["nc"] = _build_nc()
    trace = bool(int(os.environ.get("KERNEL_TRACE", "0")))
    res = run_bass_kernel_spmd(
        _CACHE["nc"], in_maps, core_ids=list(range(NCORES)), trace=trace,
    )
    _CACHE["last_result"] = res
    out = np.concatenate([res.results[i]["out"] for i in range(NCORES)], axis=0)
    return out.reshape(1, S, L, 256)
